# revision 25
# baseline (speedup 1.0000x reference)
"""DeepseekMoE block-quantized MoE kernel for 8 Trainium2 NeuronCores.

Strategy (expert-parallel with host-side dispatch):
  - The routing table (selected_experts) is known on the host before launch,
    so the all-to-all "dispatch" is done on the host: for each expert e we
    gather the unique tokens routed to it (dedup across the top-k slots),
    transpose to [H, n_e], and pad to a common capacity C.
  - Experts are sharded 2-per-core across the 8 cores.  Each core runs a
    dense 3-matmul MLP (gate/up -> silu*up -> down) for its 2 experts in
    x^T / act^T layout so no on-device transposes are needed.
  - Block-dequantization (w * repeat(s, 128)) is folded into the host-side
    weight preparation, which also rounds weights and x to bf16.
  - bf16 matmuls stream 1 column/cycle and enable Fast Weight Load
    (LDWEIGHTS ~53ns vs ~187ns for fp32r), so the stationary reload fully
    hides behind the moving-operand stream.  Accuracy: ~4.4e-3 rel L2
    against the fp32 reference (tolerance 2e-2).
  - Weights live in DRAM pre-swizzled into the exact SBUF slab layout so
    each slab load is one contiguous-per-partition DMA (4KB descriptors).
  - All input DMAs ride the sync queue in chunk-major need order (~120KB
    granules).  Transfers execute in strict FIFO order per queue and the
    queues arbitrate fairly, so a second queue would steal bandwidth from
    the critical stream.
  - The host scatters the per-expert outputs back to [T, K, H].
"""

import math

import numpy as np

T = 4096
TOPK = 6
E = 16
H = 2048
I = 1408
BS = 128           # quant block size
HT = H // 128      # 16 h-tiles
IT = I // 128      # 11 i-tiles
NCORES = 8
# Single-pass SBUF budget bound: (HT + IT) * 2 * W bytes of x+act per
# partition plus ~50KB of weight/output staging must fit in ~208KB.
MAX_W = 2880

_BUILT = {}
LAST_RESULTS = None  # stashed BassKernelResults for external harnesses


def _chunk_plan(width):
    """Split `width` columns into PSUM-bank-sized chunks (<=512)."""
    if width <= 512:
        return [(0, width)]
    n = -(-width // 512)
    # 8-aligned chunk widths
    base = (width // n) // 8 * 8
    rem8 = (width - n * base) // 8
    out, off = [], 0
    for j in range(n):
        w = base + (8 if j < rem8 else 0)
        if j == n - 1:
            w = width - off
        out.append((off, w))
        off += w
    return out


def _build(jobs, CT):
    """Build the SPMD Bass program.  `jobs` is a tuple of
    (slot, col_offset, width): each job runs one expert slot's MLP over a
    window of `width` token columns; CT is the column capacity of xt/yt."""
    import concourse.bacc as bacc
    import concourse.mybir as mybir
    from concourse.bass import ts
    from concourse.tile import TileContext

    f32 = mybir.dt.float32
    bf16 = mybir.dt.bfloat16
    AF = mybir.ActivationFunctionType
    import os as _os

    act_fn = (
        AF.Sigmoid if _os.environ.get("KERNEL_SIM_SIGMOID") else AF.Silu
    )  # CoreSim lacks Silu; HW path always uses Silu

    nc = bacc.Bacc()
    xt = nc.declare_dram_parameter("xt", [2, HT, 128, CT], bf16, isOutput=False)
    # slab layouts: w0t[s, i, p, h*128+j] = W0deq[i*128+j, h*128+p]
    #               w2t[s, h, p, i*128+j] = W2deq[h*128+j, i*128+p]
    w0t = nc.declare_dram_parameter("w0t", [2, IT, 128, H], bf16, isOutput=False)
    w1t = nc.declare_dram_parameter("w1t", [2, IT, 128, H], bf16, isOutput=False)
    w2t = nc.declare_dram_parameter("w2t", [2, HT, 128, I], bf16, isOutput=False)
    yt = nc.declare_dram_parameter("yt", [2, HT, 128, CT], bf16, isOutput=True)

    with TileContext(nc) as tc:
        with (
            tc.tile_pool(name="xp", bufs=1) as xp,
            tc.tile_pool(name="ap", bufs=1) as apool,
            tc.tile_pool(name="wp", bufs=2) as wp,
            tc.tile_pool(name="yp", bufs=4) as yp,
            tc.tile_pool(name="ps", bufs=2, space="PSUM") as ps,
        ):
            def load_w01_slab(which, src, s, i, graded=False):
                slab = wp.tile([128, H], bf16, tag=which, name=None)
                if graded:
                    # Prefix pieces so the first LDWEIGHTS only waits on
                    # the first 128 columns, not the whole slab.
                    for off, ln in ((0, 128), (128, 384), (512, 512), (1024, 1024)):
                        nc.sync.dma_start(
                            out=slab[:, off : off + ln],
                            in_=src[s, i, :, off : off + ln],
                        )
                else:
                    nc.sync.dma_start(out=slab, in_=src[s, i])
                return slab

            for jn, (s, co, W) in enumerate(jobs):
                    chunks = _chunk_plan(W)
                    xs = [
                        xp.tile([128, W], bf16, tag=f"x{h}", name=f"x{h}_{jn}")
                        for h in range(HT)
                    ]
                    # Bandwidth-priority emission: the i=0 gate slab first
                    # (graded on the first job so the PE can start within
                    # ~1us of the DMA stream); x chunks in c-major order
                    # gate the first matmul groups on ~1/nch of the x bytes.
                    # Job 0's startup is DMA-bandwidth-bound: split every
                    # granule stream h-even/h-odd across the sync and
                    # scalar queues so both carry identically need-ordered
                    # halves (~2x fill rate).  Later jobs prefetch during
                    # the previous job's phase B where sync alone has
                    # plenty of slack.
                    w0s_first = load_w01_slab("w0", w0t, s, 0, graded=(jn == 0))
                    for ci, (c0, cw) in enumerate(chunks):
                        for h in range(HT):
                            eng = nc.scalar if (jn == 0 and h % 2) else nc.sync
                            eng.dma_start(
                                out=xs[h][:, c0 : c0 + cw],
                                in_=xt[s, h, :, co + c0 : co + c0 + cw],
                            )
                        if ci == 0:
                            if jn == 0:
                                w1s_first = wp.tile([128, H], bf16, tag="w1")
                                nc.sync.dma_start(
                                    out=w1s_first[:, 0:1024],
                                    in_=w1t[s, 0, :, 0:1024],
                                )
                                nc.scalar.dma_start(
                                    out=w1s_first[:, 1024:H],
                                    in_=w1t[s, 0, :, 1024:H],
                                )
                            else:
                                w1s_first = load_w01_slab("w1", w1t, s, 0)
                    acts = [
                        apool.tile([128, W], bf16, tag=f"a{i}", name=f"a{i}_{jn}")
                        for i in range(IT)
                    ]

                    # Phase A: gate/up projections + silu*up, per i-tile.
                    for i in range(IT):
                        if i == 0:
                            w0s, w1s = w0s_first, w1s_first
                        else:
                            w0s = load_w01_slab("w0", w0t, s, i)
                            w1s = load_w01_slab("w1", w1t, s, i)
                        for c0, cw in chunks:
                            g = ps.tile([128, 512], f32, tag="g")
                            for h in range(HT):
                                nc.tensor.matmul(
                                    g[:, :cw],
                                    w0s[:, ts(h, 128)],
                                    xs[h][:, c0 : c0 + cw],
                                    start=(h == 0),
                                    stop=(h == HT - 1),
                                )
                            u = ps.tile([128, 512], f32, tag="u")
                            for h in range(HT):
                                nc.tensor.matmul(
                                    u[:, :cw],
                                    w1s[:, ts(h, 128)],
                                    xs[h][:, c0 : c0 + cw],
                                    start=(h == 0),
                                    stop=(h == HT - 1),
                                )
                            a_sl = acts[i][:, c0 : c0 + cw]
                            nc.scalar.activation(a_sl, g[:, :cw], act_fn)
                            nc.vector.tensor_mul(a_sl, a_sl, u[:, :cw])

                    # Phase B: down projection, per h-tile, chunk-inner so
                    # each stationary loads once per 3 matmuls instead of
                    # per matmul (saves the LDWEIGHTS NX dispatch slot).
                    # o rotates 4 PSUM banks (2g+2u+4o = 8) so the copies
                    # have a full h-tile of drain slack.
                    # w2 slabs are emitted 3 h-tiles ahead of use so
                    # their transfers sit before the copy-gated y stores
                    # in the sync queue's strict-FIFO transfer stream.
                    def emit_w2(hh, jn=jn, s=s):
                        t = wp.tile(
                            [128, I], bf16, tag="w2", bufs=5,
                            name=f"w2_{jn}_{hh}",
                        )
                        nc.sync.dma_start(out=t, in_=w2t[s, hh])
                        return t

                    w2_tiles = [emit_w2(hh) for hh in range(min(3, HT))]
                    for h in range(HT):
                        w2s = w2_tiles[h]
                        os_ = [
                            ps.tile([128, 512], f32, tag="o", bufs=4,
                                    name=f"o{ci}_{jn}_{h}")
                            for ci in range(len(chunks))
                        ]
                        for i in range(IT):
                            for (c0, cw), o in zip(chunks, os_):
                                nc.tensor.matmul(
                                    o[:, :cw],
                                    w2s[:, ts(i, 128)],
                                    acts[i][:, c0 : c0 + cw],
                                    start=(i == 0),
                                    stop=(i == IT - 1),
                                )
                        if h + 3 < HT:
                            w2_tiles.append(emit_w2(h + 3))
                        for (c0, cw), o in zip(chunks, os_):
                            yc = yp.tile([128, 512], bf16, tag="y")
                            nc.vector.tensor_copy(yc[:, :cw], o[:, :cw])
                            nc.sync.dma_start(
                                out=yt[s, h, :, co + c0 : co + c0 + cw],
                                in_=yc[:, :cw],
                            )
    nc.finalize()
    return nc


def _get_built(jobs, CT):
    key = (tuple(jobs), CT)
    if key not in _BUILT:
        _BUILT[key] = _build(tuple(jobs), CT)
    return _BUILT[key]


def _dequant(w, s):
    """w: [E, O, Iin], s: [E, O, Iin//128] -> dequantized [E, O, Iin]."""
    e, o, iin = w.shape
    return (w.reshape(e, o, iin // BS, BS) * s[..., None]).reshape(e, o, iin)


def kernel(**inputs):
    global LAST_RESULTS
    import ml_dtypes

    bf16 = ml_dtypes.bfloat16

    x = np.ascontiguousarray(np.asarray(inputs["x"], dtype=np.float32))
    sel = np.asarray(inputs["selected_experts"])
    w0 = np.asarray(inputs["w0"], dtype=np.float32)
    s0 = np.asarray(inputs["s0"], dtype=np.float32)
    w1 = np.asarray(inputs["w1"], dtype=np.float32)
    s1 = np.asarray(inputs["s1"], dtype=np.float32)
    w2 = np.asarray(inputs["w2"], dtype=np.float32)
    s2 = np.asarray(inputs["s2"], dtype=np.float32)

    t, k = sel.shape
    assert (t, k) == (T, TOPK) and x.shape == (T, H)

    # ---- host-side dispatch: unique tokens per expert ----
    pos = np.full((E, T), -1, dtype=np.int32)
    cols = []
    for e in range(E):
        toks = np.nonzero((sel == e).any(axis=1))[0]
        cols.append(toks)
        pos[e, toks] = np.arange(len(toks), dtype=np.int32)
    counts = np.array([len(c) for c in cols])

    # Assign experts to (core, slot): slot 0 holds the 8 largest experts,
    # slot 1 the 8 smallest, so each slot's padded width is only the max of
    # its own rank group.  expert_of[s][c] = expert on core c, slot s.
    order = np.argsort(-counts, kind="stable")
    expert_of = [list(order[:NCORES]), list(order[NCORES:])]

    def align4(v):
        return max(256, -(-v // 4) * 4)

    slot_w = [align4(int(counts[expert_of[s]].max())) for s in range(2)]

    if max(slot_w) <= MAX_W:
        jobs = tuple((s, 0, slot_w[s]) for s in range(2))
        CT = max(slot_w)
    else:
        # fallback: uniform width, multiple column windows per slot
        cmax = int(counts.max())
        passes = max(1, math.ceil(cmax / MAX_W))
        W = align4(math.ceil(cmax / passes))
        CT = W * passes
        jobs = tuple((s, cp * W, W) for s in range(2) for cp in range(passes))

    # ---- dequantize + swizzle weights into SBUF slab layout (host) ----
    #   w0slab[e, it, p, ht*128+jj] = W0deq[e, it*128+jj, ht*128+p]
    #   w2slab[e, ht, p, it*128+jj] = W2deq[e, ht*128+jj, it*128+p]
    W0d = _dequant(w0, s0)  # [E, I, H]
    W1d = _dequant(w1, s1)  # [E, I, H]
    W2d = _dequant(w2, s2)  # [E, H, I]
    w0slab = np.ascontiguousarray(
        W0d.reshape(E, IT, 128, HT, 128).transpose(0, 1, 4, 3, 2)
    ).reshape(E, IT, 128, H).astype(bf16)
    w1slab = np.ascontiguousarray(
        W1d.reshape(E, IT, 128, HT, 128).transpose(0, 1, 4, 3, 2)
    ).reshape(E, IT, 128, H).astype(bf16)
    w2slab = np.ascontiguousarray(
        W2d.reshape(E, HT, 128, IT, 128).transpose(0, 1, 4, 3, 2)
    ).reshape(E, HT, 128, I).astype(bf16)

    xb = x.astype(bf16)

    in_maps = []
    for c in range(NCORES):
        pair = [expert_of[0][c], expert_of[1][c]]
        xt_c = np.zeros((2, H, CT), dtype=bf16)
        for s, e in enumerate(pair):
            n = len(cols[e])
            if n:
                xt_c[s, :, :n] = xb[cols[e]].T
        in_maps.append(
            {
                "xt": xt_c.reshape(2, HT, 128, CT),
                "w0t": w0slab[pair],
                "w1t": w1slab[pair],
                "w2t": w2slab[pair],
            }
        )

    nc = _get_built(jobs, CT)
    from concourse.bass_utils import run_bass_kernel_spmd

    res = run_bass_kernel_spmd(nc, in_maps, list(range(NCORES)))
    LAST_RESULTS = res

    # Y[e] = [H, CT] for expert e
    Y = np.empty((E, H, CT), dtype=np.float32)
    for c in range(NCORES):
        yt_c = np.asarray(res.results[c]["yt"]).astype(np.float32).reshape(2, H, CT)
        Y[expert_of[0][c]] = yt_c[0]
        Y[expert_of[1][c]] = yt_c[1]

    # ---- scatter back to [T, K, H] ----
    e_flat = sel.reshape(-1).astype(np.int64)
    t_flat = np.repeat(np.arange(T, dtype=np.int64), TOPK)
    p_flat = pos[e_flat, t_flat]
    out = Y[e_flat, :, p_flat]  # [T*K, H]
    return np.ascontiguousarray(out.reshape(T, TOPK, H), dtype=np.float32)


# revision 27
# speedup vs baseline: 1.1969x; 1.1969x over previous
"""DeepseekMoE block-quantized MoE kernel for 8 Trainium2 NeuronCores.

Strategy (expert-parallel with host-side dispatch):
  - The routing table (selected_experts) is known on the host before launch,
    so the all-to-all "dispatch" is done on the host: for each expert e we
    gather the unique tokens routed to it (dedup across the top-k slots),
    transpose to [H, n_e], and pad to a common capacity C.
  - Experts are sharded 2-per-core across the 8 cores.  Each core runs a
    dense 3-matmul MLP (gate/up -> silu*up -> down) for its 2 experts in
    x^T / act^T layout so no on-device transposes are needed.
  - Block-dequantization (w * repeat(s, 128)) is folded into the host-side
    weight preparation, which also rounds weights and x to bf16.
  - bf16 matmuls stream 1 column/cycle and enable Fast Weight Load
    (LDWEIGHTS ~53ns vs ~187ns for fp32r), so the stationary reload fully
    hides behind the moving-operand stream.  Accuracy: ~4.4e-3 rel L2
    against the fp32 reference (tolerance 2e-2).
  - Weights live in DRAM pre-swizzled into the exact SBUF slab layout so
    each slab load is one contiguous-per-partition DMA (4KB descriptors).
  - All input DMAs ride the sync queue in chunk-major need order (~120KB
    granules).  Transfers execute in strict FIFO order per queue and the
    queues arbitrate fairly, so a second queue would steal bandwidth from
    the critical stream.
  - The host scatters the per-expert outputs back to [T, K, H].
"""

import math

import numpy as np

T = 4096
TOPK = 6
E = 16
H = 2048
I = 1408
BS = 128           # quant block size
HT = H // 128      # 16 h-tiles
IT = I // 128      # 11 i-tiles
NCORES = 8
# Single-pass SBUF budget bound: (HT + IT) * 2 * W bytes of x+act per
# partition plus ~50KB of weight/output staging must fit in ~208KB.
MAX_W = 2880

_BUILT = {}
LAST_RESULTS = None  # stashed BassKernelResults for external harnesses


def _chunk_plan(width):
    """Split `width` columns into PSUM-bank-sized chunks (<=512)."""
    if width <= 512:
        return [(0, width)]
    n = -(-width // 512)
    # 8-aligned chunk widths
    base = (width // n) // 8 * 8
    rem8 = (width - n * base) // 8
    out, off = [], 0
    for j in range(n):
        w = base + (8 if j < rem8 else 0)
        if j == n - 1:
            w = width - off
        out.append((off, w))
        off += w
    return out


def _build(jobs, CT):
    """Build the SPMD Bass program.  `jobs` is a tuple of
    (slot, col_offset, width): each job runs one expert slot's MLP over a
    window of `width` token columns; CT is the column capacity of xt/yt."""
    import concourse.bacc as bacc
    import concourse.mybir as mybir
    from concourse.bass import ts
    from concourse.tile import TileContext

    f32 = mybir.dt.float32
    bf16 = mybir.dt.bfloat16
    AF = mybir.ActivationFunctionType
    import os as _os

    act_fn = (
        AF.Sigmoid if _os.environ.get("KERNEL_SIM_SIGMOID") else AF.Silu
    )  # CoreSim lacks Silu; HW path always uses Silu

    nc = bacc.Bacc()
    xt = nc.declare_dram_parameter("xt", [2, HT, 128, CT], bf16, isOutput=False)
    # slab layouts: w0t[s, i, p, h*128+j] = W0deq[i*128+j, h*128+p]
    #               w2t[s, h, p, i*128+j] = W2deq[h*128+j, i*128+p]
    w0t = nc.declare_dram_parameter("w0t", [2, IT, 128, H], bf16, isOutput=False)
    w1t = nc.declare_dram_parameter("w1t", [2, IT, 128, H], bf16, isOutput=False)
    w2t = nc.declare_dram_parameter("w2t", [2, HT, 128, I], bf16, isOutput=False)
    yt = nc.declare_dram_parameter("yt", [2, HT, 128, CT], f32, isOutput=True)

    with TileContext(nc) as tc:
        with (
            tc.tile_pool(name="xp", bufs=1) as xp,
            tc.tile_pool(name="ap", bufs=1) as apool,
            tc.tile_pool(name="wp", bufs=2) as wp,
            tc.tile_pool(name="yp", bufs=4) as yp,
            tc.tile_pool(name="ps", bufs=2, space="PSUM") as ps,
        ):
            def load_w01_slab(which, src, s, i, graded=False):
                slab = wp.tile([128, H], bf16, tag=which, name=None)
                if graded:
                    # Prefix pieces so the first LDWEIGHTS only waits on
                    # the first 128 columns, not the whole slab.
                    for off, ln in ((0, 128), (128, 384), (512, 512), (1024, 1024)):
                        nc.sync.dma_start(
                            out=slab[:, off : off + ln],
                            in_=src[s, i, :, off : off + ln],
                        )
                else:
                    nc.sync.dma_start(out=slab, in_=src[s, i])
                return slab

            for jn, (s, co, W) in enumerate(jobs):
                    chunks = _chunk_plan(W)
                    xs = [
                        xp.tile([128, W], bf16, tag=f"x{h}", name=f"x{h}_{jn}")
                        for h in range(HT)
                    ]
                    # Bandwidth-priority emission: the i=0 gate slab first
                    # (graded on the first job so the PE can start within
                    # ~1us of the DMA stream); x chunks in c-major order
                    # gate the first matmul groups on ~1/nch of the x bytes.
                    # Job 0's startup is DMA-bandwidth-bound: split every
                    # granule stream h-even/h-odd across the sync and
                    # scalar queues so both carry identically need-ordered
                    # halves (~2x fill rate).  Later jobs prefetch during
                    # the previous job's phase B where sync alone has
                    # plenty of slack.
                    w0s_first = load_w01_slab("w0", w0t, s, 0, graded=(jn == 0))
                    for ci, (c0, cw) in enumerate(chunks):
                        for h in range(HT):
                            eng = nc.scalar if (jn == 0 and h % 2) else nc.sync
                            eng.dma_start(
                                out=xs[h][:, c0 : c0 + cw],
                                in_=xt[s, h, :, co + c0 : co + c0 + cw],
                            )
                        if ci == 0:
                            if jn == 0:
                                w1s_first = wp.tile([128, H], bf16, tag="w1")
                                nc.sync.dma_start(
                                    out=w1s_first[:, 0:1024],
                                    in_=w1t[s, 0, :, 0:1024],
                                )
                                nc.scalar.dma_start(
                                    out=w1s_first[:, 1024:H],
                                    in_=w1t[s, 0, :, 1024:H],
                                )
                            else:
                                w1s_first = load_w01_slab("w1", w1t, s, 0)
                    acts = [
                        apool.tile([128, W], bf16, tag=f"a{i}", name=f"a{i}_{jn}")
                        for i in range(IT)
                    ]

                    # Phase A: gate/up projections + silu*up, per i-tile.
                    for i in range(IT):
                        if i == 0:
                            w0s, w1s = w0s_first, w1s_first
                        else:
                            w0s = load_w01_slab("w0", w0t, s, i)
                            w1s = load_w01_slab("w1", w1t, s, i)
                        for c0, cw in chunks:
                            g = ps.tile([128, 512], f32, tag="g")
                            for h in range(HT):
                                nc.tensor.matmul(
                                    g[:, :cw],
                                    w0s[:, ts(h, 128)],
                                    xs[h][:, c0 : c0 + cw],
                                    start=(h == 0),
                                    stop=(h == HT - 1),
                                )
                            u = ps.tile([128, 512], f32, tag="u")
                            for h in range(HT):
                                nc.tensor.matmul(
                                    u[:, :cw],
                                    w1s[:, ts(h, 128)],
                                    xs[h][:, c0 : c0 + cw],
                                    start=(h == 0),
                                    stop=(h == HT - 1),
                                )
                            a_sl = acts[i][:, c0 : c0 + cw]
                            nc.scalar.activation(a_sl, g[:, :cw], act_fn)
                            nc.vector.tensor_mul(a_sl, a_sl, u[:, :cw])

                    # Phase B: down projection, per h-tile, chunk-inner so
                    # each stationary loads once per 3 matmuls instead of
                    # per matmul (saves the LDWEIGHTS NX dispatch slot).
                    # o rotates 4 PSUM banks (2g+2u+4o = 8) so the copies
                    # have a full h-tile of drain slack.
                    for h in range(HT):
                        w2s = wp.tile([128, I], bf16, tag="w2", bufs=3)
                        nc.sync.dma_start(out=w2s, in_=w2t[s, h])
                        if jn == len(jobs) - 1 and h == HT - 1:
                            # Final h-tile: chunk-outer so the c0/c1 copies
                            # and stores overlap the c1/c2 matmuls instead
                            # of trailing the very last matmul (-1.8us
                            # tail, verified in an earlier trace).
                            for c0, cw in chunks:
                                o = ps.tile([128, 512], f32, tag="o", bufs=4)
                                for i in range(IT):
                                    nc.tensor.matmul(
                                        o[:, :cw],
                                        w2s[:, ts(i, 128)],
                                        acts[i][:, c0 : c0 + cw],
                                        start=(i == 0),
                                        stop=(i == IT - 1),
                                    )
                                yc = yp.tile([128, 512], f32, tag="y")
                                nc.vector.tensor_copy(yc[:, :cw], o[:, :cw])
                                nc.sync.dma_start(
                                    out=yt[s, h, :, co + c0 : co + c0 + cw],
                                    in_=yc[:, :cw],
                                )
                            continue
                        os_ = [
                            ps.tile([128, 512], f32, tag="o", bufs=4,
                                    name=f"o{ci}_{jn}_{h}")
                            for ci in range(len(chunks))
                        ]
                        for i in range(IT):
                            for (c0, cw), o in zip(chunks, os_):
                                nc.tensor.matmul(
                                    o[:, :cw],
                                    w2s[:, ts(i, 128)],
                                    acts[i][:, c0 : c0 + cw],
                                    start=(i == 0),
                                    stop=(i == IT - 1),
                                )
                        for (c0, cw), o in zip(chunks, os_):
                            yc = yp.tile([128, 512], f32, tag="y")
                            nc.vector.tensor_copy(yc[:, :cw], o[:, :cw])
                            nc.sync.dma_start(
                                out=yt[s, h, :, co + c0 : co + c0 + cw],
                                in_=yc[:, :cw],
                            )
    nc.finalize()
    return nc


def _get_built(jobs, CT):
    key = (tuple(jobs), CT)
    if key not in _BUILT:
        _BUILT[key] = _build(tuple(jobs), CT)
    return _BUILT[key]


def _dequant(w, s):
    """w: [E, O, Iin], s: [E, O, Iin//128] -> dequantized [E, O, Iin]."""
    e, o, iin = w.shape
    return (w.reshape(e, o, iin // BS, BS) * s[..., None]).reshape(e, o, iin)


def kernel(**inputs):
    global LAST_RESULTS
    import ml_dtypes

    bf16 = ml_dtypes.bfloat16

    x = np.ascontiguousarray(np.asarray(inputs["x"], dtype=np.float32))
    sel = np.asarray(inputs["selected_experts"])
    w0 = np.asarray(inputs["w0"], dtype=np.float32)
    s0 = np.asarray(inputs["s0"], dtype=np.float32)
    w1 = np.asarray(inputs["w1"], dtype=np.float32)
    s1 = np.asarray(inputs["s1"], dtype=np.float32)
    w2 = np.asarray(inputs["w2"], dtype=np.float32)
    s2 = np.asarray(inputs["s2"], dtype=np.float32)

    t, k = sel.shape
    assert (t, k) == (T, TOPK) and x.shape == (T, H)

    # ---- host-side dispatch: unique tokens per expert ----
    pos = np.full((E, T), -1, dtype=np.int32)
    cols = []
    for e in range(E):
        toks = np.nonzero((sel == e).any(axis=1))[0]
        cols.append(toks)
        pos[e, toks] = np.arange(len(toks), dtype=np.int32)
    counts = np.array([len(c) for c in cols])

    # Assign experts to (core, slot): slot 0 holds the 8 largest experts,
    # slot 1 the 8 smallest, so each slot's padded width is only the max of
    # its own rank group.  expert_of[s][c] = expert on core c, slot s.
    order = np.argsort(-counts, kind="stable")
    expert_of = [list(order[:NCORES]), list(order[NCORES:])]

    def align4(v):
        return max(256, -(-v // 4) * 4)

    slot_w = [align4(int(counts[expert_of[s]].max())) for s in range(2)]

    if max(slot_w) <= MAX_W:
        jobs = tuple((s, 0, slot_w[s]) for s in range(2))
        CT = max(slot_w)
    else:
        # fallback: uniform width, multiple column windows per slot
        cmax = int(counts.max())
        passes = max(1, math.ceil(cmax / MAX_W))
        W = align4(math.ceil(cmax / passes))
        CT = W * passes
        jobs = tuple((s, cp * W, W) for s in range(2) for cp in range(passes))

    # ---- dequantize + swizzle weights into SBUF slab layout (host) ----
    #   w0slab[e, it, p, ht*128+jj] = W0deq[e, it*128+jj, ht*128+p]
    #   w2slab[e, ht, p, it*128+jj] = W2deq[e, ht*128+jj, it*128+p]
    W0d = _dequant(w0, s0)  # [E, I, H]
    W1d = _dequant(w1, s1)  # [E, I, H]
    W2d = _dequant(w2, s2)  # [E, H, I]
    w0slab = np.ascontiguousarray(
        W0d.reshape(E, IT, 128, HT, 128).transpose(0, 1, 4, 3, 2)
    ).reshape(E, IT, 128, H).astype(bf16)
    w1slab = np.ascontiguousarray(
        W1d.reshape(E, IT, 128, HT, 128).transpose(0, 1, 4, 3, 2)
    ).reshape(E, IT, 128, H).astype(bf16)
    w2slab = np.ascontiguousarray(
        W2d.reshape(E, HT, 128, IT, 128).transpose(0, 1, 4, 3, 2)
    ).reshape(E, HT, 128, I).astype(bf16)

    xb = x.astype(bf16)

    in_maps = []
    for c in range(NCORES):
        pair = [expert_of[0][c], expert_of[1][c]]
        xt_c = np.zeros((2, H, CT), dtype=bf16)
        for s, e in enumerate(pair):
            n = len(cols[e])
            if n:
                xt_c[s, :, :n] = xb[cols[e]].T
        in_maps.append(
            {
                "xt": xt_c.reshape(2, HT, 128, CT),
                "w0t": w0slab[pair],
                "w1t": w1slab[pair],
                "w2t": w2slab[pair],
            }
        )

    nc = _get_built(jobs, CT)
    from concourse.bass_utils import run_bass_kernel_spmd

    res = run_bass_kernel_spmd(nc, in_maps, list(range(NCORES)))
    LAST_RESULTS = res

    # Y[e] = [H, CT] for expert e
    Y = np.empty((E, H, CT), dtype=np.float32)
    for c in range(NCORES):
        yt_c = np.asarray(res.results[c]["yt"]).reshape(2, H, CT)
        Y[expert_of[0][c]] = yt_c[0]
        Y[expert_of[1][c]] = yt_c[1]

    # ---- scatter back to [T, K, H] ----
    e_flat = sel.reshape(-1).astype(np.int64)
    t_flat = np.repeat(np.arange(T, dtype=np.int64), TOPK)
    p_flat = pos[e_flat, t_flat]
    out = Y[e_flat, :, p_flat]  # [T*K, H]
    return np.ascontiguousarray(out.reshape(T, TOPK, H), dtype=np.float32)
